# revision 13
# baseline (speedup 1.0000x reference)
"""Circular relative-position attention on 8 trn2 NeuronCores.

Algorithm (per (batch,head), S=1024, hd=64):
  scores[q,k] = dot(Q[q],K[k])/8 + dot(Q[q], Wk[(k-q)%S]),  Wk[u] = rel_pos_k[min(u,S-u)]
  attn = softmax_k(scores)
  out[q] = sum_k attn[q,k]*V[k] + sum_u attn[q,(q+u)%S]*Wv[u],  Wv[u] = rel_pos_v[min(u,S-u)]

v7 layout (all-on-chip, no DRAM round-trips):
  - Phase A: skew[q,u] = Q @ (8*Wk)^T per q-tile -> DVE cast to bf16 relb
    [128, S+128] (wrap band = cols 0:128).
  - Phase B: scores psum = Q@K^T; rb (sheared rel) read from relb via
    *diagonal SBUF->SBUF DMAs* (per-partition offset -i); added into psum
    with an identity matmul; ACT exp(0.125 x) -> pexp bf16 [128, S+128]
    with accum_out row sums (denominators).  PS = pre-skewed pexp
    (diagonal DMA, shift +i) for the rel-V stream.
  - Phase C: attn^T streams built by PE transposes (128x128 blocks) into
    bf16 PSUM, copied to SBUF (DVE/Pool), then out^T[d,q] psum accumulates
    V^T@attnT + Wv^T@attnT_skew over k-tiles.
  - Finalize: out^T copied to SBUF bf16, PE-transposed back to [q,d],
    scaled by 1/denominator (per-partition) on DVE, stored as [q, d] f32.
  - batch is sharded across the 8 cores (1 batch each, 16 heads).
"""

import os
import sys
import numpy as np

for _p in ("/opt/trn_rl_repo", "/root/.axon_site/_ro/trn_rl_repo"):
    if os.path.isdir(_p) and _p not in sys.path:
        sys.path.insert(0, _p)

import ml_dtypes
from contextlib import ExitStack

import concourse.bass as bass
import concourse.tile as tile
from concourse import bacc, mybir
from concourse.masks import make_identity

FP32 = mybir.dt.float32
F32R = mybir.dt.float32r
BF16 = mybir.dt.bfloat16

B, S, D, H = 8, 1024, 1024, 16
HD = D // H
NCORES = 8


def build_module(nbh=H, s=S, hd=HD, n_act_casts=4):
    """Per-core Bass module. nbh (b,h) pairs, seq len s, head dim hd."""
    nt = s // 128            # q/k tiles
    ch = 512                 # matmul free-dim chunk (one psum bank)
    we = s + 128             # relb/pexp width (wrap band)
    nc = bacc.Bacc("TRN2", target_bir_lowering=False, debug=False)

    qT = nc.dram_tensor("qT", [nbh, hd, s], F32R, kind="ExternalInput")
    kT = nc.dram_tensor("kT", [nbh, hd, s], F32R, kind="ExternalInput")
    v = nc.dram_tensor("v", [nbh, s, hd], BF16, kind="ExternalInput")
    wT = nc.dram_tensor("wT", [hd, s], F32R, kind="ExternalInput")
    wv = nc.dram_tensor("wv", [s, hd], BF16, kind="ExternalInput")
    out = nc.dram_tensor("out", [nbh, s, hd], FP32, kind="ExternalOutput")
    attq3 = nc.dram_tensor("attq3", [nbh, s // 128, 128, s + 128], BF16,
                           kind="Internal")

    def dap(tensor, offset, pattern):
        return bass.AP(tensor, offset, pattern)

    def tile_ap(t, offset, pattern):
        return bass.AP(t[:].tensor, t[:].offset + offset, pattern)

    with tile.TileContext(nc) as tc, ExitStack() as ctx:
        const_pool = ctx.enter_context(tc.tile_pool(name="const", bufs=1))
        qkv_pool = ctx.enter_context(tc.tile_pool(name="qkv", bufs=2))
        relb_pool = ctx.enter_context(tc.tile_pool(name="relb", bufs=nt + 1))
        pexp_pool = ctx.enter_context(tc.tile_pool(name="pexp", bufs=nt + 2))
        rb_pool = ctx.enter_context(tc.tile_pool(name="rb", bufs=3))
        dst_pool = ctx.enter_context(tc.tile_pool(name="dst", bufs=nt + 1))
        str_pool = ctx.enter_context(tc.tile_pool(name="strm", bufs=4))
        sm_pool = ctx.enter_context(tc.tile_pool(name="small", bufs=4))
        dnt_pool = ctx.enter_context(tc.tile_pool(name="dnt", bufs=2 * nt + 2))
        score_ps = ctx.enter_context(
            tc.tile_pool(name="score", bufs=2, space="PSUM"))
        tp_ps = ctx.enter_context(tc.tile_pool(name="tp", bufs=2, space="PSUM"))
        out_ps = ctx.enter_context(tc.tile_pool(name="po", bufs=1, space="PSUM"))

        tp_sem = nc.alloc_semaphore("tp_sem")
        ident = const_pool.tile([128, 128], BF16)
        make_identity(nc, ident[:])
        identf = const_pool.tile([128, 128], FP32)
        make_identity(nc, identf[:])
        wT_sb = const_pool.tile([hd, s], F32R)
        nc.sync.dma_start(wT_sb[:], wT.ap())
        wv_sb = const_pool.tile([128, nt * hd], BF16)
        nc.sync.dma_start(
            wv_sb[:], dap(wv, 0, [[hd, 128], [128 * hd, nt], [1, hd]]))

        for bh in range(nbh):
            qT_sb = qkv_pool.tile([hd, s], F32R, tag="qT")
            nc.sync.dma_start(qT_sb[:], qT.ap()[bh])
            kT_sb = qkv_pool.tile([hd, s], F32R, tag="kT")
            nc.sync.dma_start(kT_sb[:], kT.ap()[bh])
            v_sb = qkv_pool.tile([128, nt * hd], BF16, tag="v")
            nc.sync.dma_start(
                v_sb[:],
                dap(v, bh * s * hd, [[hd, 128], [128 * hd, nt], [1, hd]]))

            # ---- Phase A: skew = Q @ (8*Wk)^T, cast to bf16 with wrap band --
            relbs = []
            for t in range(nt):
                psA = score_ps.tile([128, s], FP32, tag="sc")
                lhs = qT_sb[:, t * 128:(t + 1) * 128]
                for h0 in range(0, s, ch):
                    nc.tensor.matmul(
                        psA[:, h0:h0 + ch], lhs, wT_sb[:, h0:h0 + ch],
                        start=True, stop=True)
                relb = relb_pool.tile([128, we], BF16, tag="relb")
                if t < n_act_casts:
                    nc.scalar.copy(relb[:, 0:s], psA[:])
                    nc.scalar.copy(relb[:, s:we], psA[:, 0:128])
                else:
                    nc.vector.tensor_copy(relb[:, 0:s], psA[:])
                    nc.vector.tensor_copy(relb[:, s:we], psA[:, 0:128])
                relbs.append(relb)

            # ---- Phase B: scores + rel add + exp; build pexp & PS ----
            pexps, dnts = [], []
            for t in range(nt):
                q0 = t * 128
                psB = score_ps.tile([128, s], FP32, tag="sc")
                lhs = qT_sb[:, q0:q0 + 128]
                for h0 in range(0, s, ch):
                    nc.tensor.matmul(
                        psB[:, h0:h0 + ch], lhs, kT_sb[:, h0:h0 + ch],
                        start=True, stop=False)
                relb = relbs[t]
                rb = rb_pool.tile([128, s], BF16, tag="rb")
                # rb[i,k] = relb[i, (k - q0 - i) mod s] via diagonal reads
                # read1: cols [0, q0+128): relb col = (s - q0) + k - i
                nc.gpsimd.dma_start(
                    rb[:, 0:q0 + 128],
                    tile_ap(relb, s - q0, [[we - 1, 128], [1, q0 + 128]]))
                # read2: cols [q0+128, s): relb col = 128 + m - i
                if q0 + 128 < s:
                    nc.gpsimd.dma_start(
                        rb[:, q0 + 128:s],
                        tile_ap(relb, 128, [[we - 1, 128], [1, s - q0 - 128]]))
                for h0 in range(0, s, ch):
                    nc.tensor.matmul(
                        psB[:, h0:h0 + ch], ident[:], rb[:, h0:h0 + ch],
                        start=False, stop=True)
                pexp = pexp_pool.tile([128, we], BF16, tag="pexp")
                dnt = dnt_pool.tile([128, 1], FP32, tag="dnt")
                nc.scalar.activation(
                    pexp[:, 0:s], psB[:], mybir.ActivationFunctionType.Exp,
                    scale=0.125, accum_out=dnt[:])
                nc.vector.tensor_copy(pexp[:, s:we], pexp[:, 0:128])
                # straight spill for the diagonal xbar reads in phase C
                nc.gpsimd.dma_start(
                    dap(attq3, ((bh * nt + t) * 128) * we,
                        [[we, 128], [1, we]]),
                    pexp[:])
                pexps.append(pexp)
                dnts.append(dnt)

            # ---- Phase C: attnT via PE transposes (V term) + diagonal
            # DRAM xbar reads of attq3 (rel-V term).  The xbar rides a ring
            # Tile cannot track, so gate with a manual semaphore; the wait
            # sits on the Tensor queue (the consumer).
            psO = out_ps.tile([hd, s], FP32, tag="out")
            dsts = []
            with tc.tile_critical():
                for j in range(nt):
                    dst = dst_pool.tile([128, s], BF16, tag="dst")
                    for t in range(nt):
                        g = ((j + t) % nt) * 128
                        base = ((bh * nt + t) * 128) * we
                        nc.scalar.dma_start_transpose(
                            dst[:, t * 128:(t + 1) * 128],
                            dap(attq3, base + g, [[we + 1, 128], [1, 128]]),
                        ).then_inc(tp_sem, 16)
                    dsts.append(dst)
                nc.tensor.wait_ge(tp_sem, 16 * nt * nt * (bh + 1))
            for j in range(nt):
                psTT = tp_ps.tile([128, s], BF16, tag="tp")
                for t in range(nt):
                    nc.tensor.transpose(
                        psTT[:, t * 128:(t + 1) * 128],
                        pexps[t][:, j * 128:(j + 1) * 128], ident[:])
                ptt = str_pool.tile([128, s], BF16, tag="ptt")
                nc.vector.tensor_copy(ptt[:], psTT[:])
                dst = dsts[j]
                vj = v_sb[:, j * hd:(j + 1) * hd]
                wvj = wv_sb[:, j * hd:(j + 1) * hd]
                for h0 in range(0, s, ch):
                    nc.tensor.matmul(
                        psO[:, h0:h0 + ch], vj, ptt[:, h0:h0 + ch],
                        start=(j == 0), stop=False)
                    nc.tensor.matmul(
                        psO[:, h0:h0 + ch], wvj, dst[:, h0:h0 + ch],
                        start=False, stop=(j == nt - 1 and h0 + ch >= s))

            # ---- Finalize: transpose out^T -> [q, d], scale by 1/denom ----
            outsb = sm_pool.tile([hd, s], FP32, tag="outsb")
            nc.vector.tensor_copy(outsb[:], psO[:])
            for t in range(nt):
                psF = tp_ps.tile([128, hd], FP32, tag="tp")
                nc.tensor.transpose(
                    psF[:], outsb[:, t * 128:(t + 1) * 128],
                    identf[0:hd, 0:hd])
                rec = dnt_pool.tile([128, 1], FP32, tag="rec")
                nc.vector.reciprocal(rec[:], dnts[t][:])
                res = sm_pool.tile([128, hd], FP32, tag="res")
                nc.vector.tensor_scalar_mul(res[:], psF[:], rec[:])
                nc.sync.dma_start(
                    dap(out, (bh * s + t * 128) * hd, [[hd, 128], [1, hd]]),
                    res[:])

    nc.compile()
    return nc


_NC_CACHE = {}


def _get_module(nbh, s, hd):
    key = (nbh, s, hd)
    if key not in _NC_CACHE:
        _NC_CACHE[key] = build_module(nbh, s, hd)
    return _NC_CACHE[key]


def _prep_core_inputs(query, key, value, rel_pos_k, rel_pos_v):
    """Host-side shard + layout prep. Returns per-core input maps."""
    u = np.arange(S)
    g = np.minimum(u, S - u)
    wT = (rel_pos_k[g] * 8.0).T.astype(np.float32).copy()           # (hd, S)
    wv = rel_pos_v[g].astype(ml_dtypes.bfloat16).copy()             # (S, hd)

    in_maps = []
    for c in range(NCORES):
        q_c = query[c].reshape(S, H, HD)
        k_c = key[c].reshape(S, H, HD)
        v_c = value[c].reshape(S, H, HD)
        in_maps.append({
            "qT": np.ascontiguousarray(q_c.transpose(1, 2, 0)).astype(np.float32),
            "kT": np.ascontiguousarray(k_c.transpose(1, 2, 0)).astype(np.float32),
            "v": np.ascontiguousarray(v_c.transpose(1, 0, 2)).astype(ml_dtypes.bfloat16),
            "wT": wT,
            "wv": wv,
        })
    return in_maps


def _postprocess_core(result_map):
    o = result_map["out"]                         # (H, S, HD)
    return o.transpose(1, 0, 2).reshape(S, D)


def kernel(**inputs):
    from concourse.bass_utils import run_bass_kernel_spmd

    query = np.asarray(inputs["query"], dtype=np.float32)
    key = np.asarray(inputs["key"], dtype=np.float32)
    value = np.asarray(inputs["value"], dtype=np.float32)
    rel_pos_k = np.asarray(inputs["rel_pos_k"], dtype=np.float32)
    rel_pos_v = np.asarray(inputs["rel_pos_v"], dtype=np.float32)

    nc = _get_module(H, S, HD)
    in_maps = _prep_core_inputs(query, key, value, rel_pos_k, rel_pos_v)
    res = run_bass_kernel_spmd(nc, in_maps, core_ids=list(range(NCORES)))

    outv = np.empty((B, S, D), dtype=np.float32)
    for c in range(NCORES):
        outv[c] = _postprocess_core(res.results[c])
    return outv


if __name__ == "__main__":
    rng = np.random.default_rng(0)
    ins = {
        "query": rng.standard_normal((B, S, D)).astype(np.float32),
        "key": rng.standard_normal((B, S, D)).astype(np.float32),
        "value": rng.standard_normal((B, S, D)).astype(np.float32),
        "rel_pos_k": (rng.standard_normal((S, HD)) * 0.02).astype(np.float32),
        "rel_pos_v": (rng.standard_normal((S, HD)) * 0.02).astype(np.float32),
    }
    o = kernel(**ins)
    print("out", o.shape, o.dtype, np.abs(o).max())


# revision 14
# speedup vs baseline: 2.7241x; 2.7241x over previous
"""Circular relative-position attention on 8 trn2 NeuronCores.

Algorithm (per (batch,head), S=1024, hd=64):
  scores[q,k] = dot(Q[q],K[k])/8 + dot(Q[q], Wk[(k-q)%S]),  Wk[u] = rel_pos_k[min(u,S-u)]
  attn = softmax_k(scores)
  out[q] = sum_k attn[q,k]*V[k] + sum_u attn[q,(q+u)%S]*Wv[u],  Wv[u] = rel_pos_v[min(u,S-u)]

v7.2 layout:
  - Phase A: skew[q,u] = Q @ (8*Wk)^T per q-tile -> cast to bf16 relb
    [128, S+128] (wrap band = cols 0:128) on ACT/DVE.
  - Phase B: scores psum = Q@K^T (f32r); rb (sheared rel) read from relb via
    diagonal SBUF->SBUF DMAs (per-partition shift -i; step pitch-1 is the
    one the DMA hardware honors exactly); added into psum with an identity
    matmul; ACT exp(0.125 x) -> pexp bf16 [128, S] + row sums (denoms).
    pexp is spilled straight to DRAM (main + wrap band); PS = pre-skewed
    pexp read back with a diagonal DRAM AP (byte-exact on DRAM).
  - Phase C: attn^T (V term) and attn-skew^T (rel-V term) streams built by
    PE transposes (128x128 blocks, bf16 PSUM) from pexp and PS, copied to
    SBUF by DVE, then out^T[d,q] psum accumulates V^T@attnT + Wv^T@dst.
  - Finalize: out^T -> SBUF f32, PE-transposed back to [q,d], scaled by
    1/denominator on DVE, stored as [q, d] f32.
  - The bh loop is software-pipelined: phase C of head n-1 is emitted
    between phases A/B of head n so the in-order Tensor queue always has
    independent work while ACT/DMA catch up.
  - batch is sharded across the 8 cores (1 batch each, 16 heads).
"""

import os
import sys
import numpy as np

for _p in ("/opt/trn_rl_repo", "/root/.axon_site/_ro/trn_rl_repo"):
    if os.path.isdir(_p) and _p not in sys.path:
        sys.path.insert(0, _p)

import ml_dtypes
from contextlib import ExitStack

import concourse.bass as bass
import concourse.tile as tile
from concourse import bacc, mybir
from concourse.masks import make_identity

FP32 = mybir.dt.float32
F32R = mybir.dt.float32r
BF16 = mybir.dt.bfloat16

B, S, D, H = 8, 1024, 1024, 16
HD = D // H
NCORES = 8


def build_module(nbh=H, s=S, hd=HD, n_act_casts=4):
    """Per-core Bass module. nbh (b,h) pairs, seq len s, head dim hd."""
    nt = s // 128            # q/k tiles
    ch = 512                 # matmul free-dim chunk (one psum bank)
    we = s + 128             # relb/spill width (wrap band)
    nc = bacc.Bacc("TRN2", target_bir_lowering=False, debug=False)

    qT = nc.dram_tensor("qT", [nbh, hd, s], F32R, kind="ExternalInput")
    kT = nc.dram_tensor("kT", [nbh, hd, s], F32R, kind="ExternalInput")
    v = nc.dram_tensor("v", [nbh, s, hd], BF16, kind="ExternalInput")
    wT = nc.dram_tensor("wT", [hd, s], F32R, kind="ExternalInput")
    wv = nc.dram_tensor("wv", [s, hd], BF16, kind="ExternalInput")
    out = nc.dram_tensor("out", [nbh, s, hd], FP32, kind="ExternalOutput")
    spill = nc.dram_tensor("spill", [nbh, nt, 128, we], BF16, kind="Internal")

    def dap(tensor, offset, pattern):
        return bass.AP(tensor, offset, pattern)

    def tile_ap(t, offset, pattern):
        return bass.AP(t[:].tensor, t[:].offset + offset, pattern)

    with tile.TileContext(nc) as tc, ExitStack() as ctx:
        const_pool = ctx.enter_context(tc.tile_pool(name="const", bufs=1))
        qkv_pool = ctx.enter_context(tc.tile_pool(name="qkv", bufs=2))
        relb_pool = ctx.enter_context(tc.tile_pool(name="relb", bufs=nt + 1))
        pexp_pool = ctx.enter_context(
            tc.tile_pool(name="pexp", bufs=2 * nt + 1))
        ps_pool = ctx.enter_context(tc.tile_pool(name="ps", bufs=2 * nt + 1))
        rb_pool = ctx.enter_context(tc.tile_pool(name="rb", bufs=3))
        str_pool = ctx.enter_context(tc.tile_pool(name="strm", bufs=4))
        sm_pool = ctx.enter_context(tc.tile_pool(name="small", bufs=4))
        dnt_pool = ctx.enter_context(tc.tile_pool(name="dnt", bufs=2 * nt + 2))
        score_ps = ctx.enter_context(
            tc.tile_pool(name="score", bufs=2, space="PSUM"))
        tp_ps = ctx.enter_context(tc.tile_pool(name="tp", bufs=2, space="PSUM"))
        out_ps = ctx.enter_context(tc.tile_pool(name="po", bufs=1, space="PSUM"))

        ident = const_pool.tile([128, 128], BF16)
        make_identity(nc, ident[:])
        identf = const_pool.tile([128, 128], FP32)
        make_identity(nc, identf[:])
        wT_sb = const_pool.tile([hd, s], F32R)
        nc.sync.dma_start(wT_sb[:], wT.ap())
        wv_sb = const_pool.tile([128, nt * hd], BF16)
        nc.sync.dma_start(
            wv_sb[:], dap(wv, 0, [[hd, 128], [128 * hd, nt], [1, hd]]))

        def emit_ab(bh):
            """Phases A+B for head bh; returns state for phase C."""
            qT_sb = qkv_pool.tile([hd, s], F32R, tag="qT")
            nc.sync.dma_start(qT_sb[:], qT.ap()[bh])
            kT_sb = qkv_pool.tile([hd, s], F32R, tag="kT")
            nc.sync.dma_start(kT_sb[:], kT.ap()[bh])
            v_sb = qkv_pool.tile([128, nt * hd], BF16, tag="v")
            nc.sync.dma_start(
                v_sb[:],
                dap(v, bh * s * hd, [[hd, 128], [128 * hd, nt], [1, hd]]))

            # ---- Phase A: skew = Q @ (8*Wk)^T, cast bf16 with wrap band ----
            relbs = []
            for t in range(nt):
                psA = score_ps.tile([128, s], FP32, tag="sc")
                lhs = qT_sb[:, t * 128:(t + 1) * 128]
                for h0 in range(0, s, ch):
                    nc.tensor.matmul(
                        psA[:, h0:h0 + ch], lhs, wT_sb[:, h0:h0 + ch],
                        start=True, stop=True)
                relb = relb_pool.tile([128, we], BF16, tag="relb")
                if t < n_act_casts:
                    nc.scalar.copy(relb[:, 0:s], psA[:])
                    nc.scalar.copy(relb[:, s:we], psA[:, 0:128])
                else:
                    nc.vector.tensor_copy(relb[:, 0:s], psA[:])
                    nc.vector.tensor_copy(relb[:, s:we], psA[:, 0:128])
                relbs.append(relb)

            # ---- Phase B: scores + rel add + exp; spill + PS ----
            pexps, PSs, dnts = [], [], []
            for t in range(nt):
                q0 = t * 128
                psB = score_ps.tile([128, s], FP32, tag="sc")
                lhs = qT_sb[:, q0:q0 + 128]
                for h0 in range(0, s, ch):
                    nc.tensor.matmul(
                        psB[:, h0:h0 + ch], lhs, kT_sb[:, h0:h0 + ch],
                        start=True, stop=False)
                relb = relbs[t]
                rb = rb_pool.tile([128, s], BF16, tag="rb")
                # rb[i,k] = relb[i, (k - q0 - i) mod s]: shift -i diagonals
                nc.gpsimd.dma_start(
                    rb[:, 0:q0 + 128],
                    tile_ap(relb, s - q0, [[we - 1, 128], [1, q0 + 128]]))
                if q0 + 128 < s:
                    nc.gpsimd.dma_start(
                        rb[:, q0 + 128:s],
                        tile_ap(relb, 128, [[we - 1, 128], [1, s - q0 - 128]]))
                for h0 in range(0, s, ch):
                    nc.tensor.matmul(
                        psB[:, h0:h0 + ch], ident[:], rb[:, h0:h0 + ch],
                        start=False, stop=True)
                pexp = pexp_pool.tile([128, s], BF16, tag="pexp")
                dnt = dnt_pool.tile([128, 1], FP32, tag="dnt")
                nc.scalar.activation(
                    pexp[:], psB[:], mybir.ActivationFunctionType.Exp,
                    scale=0.125, accum_out=dnt[:])
                # straight spill (main + wrap band) for the PS diagonal read
                base = ((bh * nt + t) * 128) * we
                nc.gpsimd.dma_start(
                    dap(spill, base, [[we, 128], [1, s]]), pexp[:])
                nc.gpsimd.dma_start(
                    dap(spill, base + s, [[we, 128], [1, 128]]),
                    pexp[:, 0:128])
                # PS[i,n] = pexp[i, (i+n) mod s]: diagonal DRAM read
                PSt = ps_pool.tile([128, s], BF16, tag="ps")
                nc.sync.dma_start(
                    PSt[:], dap(spill, base, [[we + 1, 128], [1, s]]))
                pexps.append(pexp)
                PSs.append(PSt)
                dnts.append(dnt)
            return v_sb, pexps, PSs, dnts

        def emit_c(bh, state):
            """Phase C + finalize for head bh."""
            v_sb, pexps, PSs, dnts = state
            psO = out_ps.tile([hd, s], FP32, tag="out")
            for j in range(nt):
                psTT = tp_ps.tile([128, s], BF16, tag="tp")
                for t in range(nt):
                    nc.tensor.transpose(
                        psTT[:, t * 128:(t + 1) * 128],
                        pexps[t][:, j * 128:(j + 1) * 128], ident[:])
                ptt = str_pool.tile([128, s], BF16, tag="ptt")
                nc.vector.tensor_copy(ptt[:], psTT[:])
                psTT2 = tp_ps.tile([128, s], BF16, tag="tp")
                for t in range(nt):
                    g = ((j + t) % nt) * 128
                    nc.tensor.transpose(
                        psTT2[:, t * 128:(t + 1) * 128],
                        PSs[t][:, g:g + 128], ident[:])
                dst = str_pool.tile([128, s], BF16, tag="dst")
                nc.vector.tensor_copy(dst[:], psTT2[:])
                vj = v_sb[:, j * hd:(j + 1) * hd]
                wvj = wv_sb[:, j * hd:(j + 1) * hd]
                for h0 in range(0, s, ch):
                    nc.tensor.matmul(
                        psO[:, h0:h0 + ch], vj, ptt[:, h0:h0 + ch],
                        start=(j == 0), stop=False)
                    nc.tensor.matmul(
                        psO[:, h0:h0 + ch], wvj, dst[:, h0:h0 + ch],
                        start=False, stop=(j == nt - 1 and h0 + ch >= s))

            # ---- Finalize: transpose out^T -> [q, d], scale by 1/denom ----
            outsb = sm_pool.tile([hd, s], FP32, tag="outsb")
            nc.vector.tensor_copy(outsb[:], psO[:])
            for t in range(nt):
                psF = tp_ps.tile([128, hd], FP32, tag="tp")
                nc.tensor.transpose(
                    psF[:], outsb[:, t * 128:(t + 1) * 128],
                    identf[0:hd, 0:hd])
                rec = dnt_pool.tile([128, 1], FP32, tag="rec")
                nc.vector.reciprocal(rec[:], dnts[t][:])
                res = sm_pool.tile([128, hd], FP32, tag="res")
                nc.vector.tensor_scalar_mul(res[:], psF[:], rec[:])
                nc.sync.dma_start(
                    dap(out, (bh * s + t * 128) * hd, [[hd, 128], [1, hd]]),
                    res[:])

        # software-pipelined bh loop
        prev = None
        for bh in range(nbh):
            state = emit_ab(bh)
            if prev is not None:
                emit_c(bh - 1, prev)
            prev = state
        emit_c(nbh - 1, prev)

    nc.compile()
    return nc


_NC_CACHE = {}


def _get_module(nbh, s, hd):
    key = (nbh, s, hd)
    if key not in _NC_CACHE:
        _NC_CACHE[key] = build_module(nbh, s, hd)
    return _NC_CACHE[key]


def _prep_core_inputs(query, key, value, rel_pos_k, rel_pos_v):
    """Host-side shard + layout prep. Returns per-core input maps."""
    u = np.arange(S)
    g = np.minimum(u, S - u)
    wT = (rel_pos_k[g] * 8.0).T.astype(np.float32).copy()           # (hd, S)
    wv = rel_pos_v[g].astype(ml_dtypes.bfloat16).copy()             # (S, hd)

    in_maps = []
    for c in range(NCORES):
        q_c = query[c].reshape(S, H, HD)
        k_c = key[c].reshape(S, H, HD)
        v_c = value[c].reshape(S, H, HD)
        in_maps.append({
            "qT": np.ascontiguousarray(q_c.transpose(1, 2, 0)).astype(np.float32),
            "kT": np.ascontiguousarray(k_c.transpose(1, 2, 0)).astype(np.float32),
            "v": np.ascontiguousarray(v_c.transpose(1, 0, 2)).astype(ml_dtypes.bfloat16),
            "wT": wT,
            "wv": wv,
        })
    return in_maps


def _postprocess_core(result_map):
    o = result_map["out"]                         # (H, S, HD)
    return o.transpose(1, 0, 2).reshape(S, D)


def kernel(**inputs):
    from concourse.bass_utils import run_bass_kernel_spmd

    query = np.asarray(inputs["query"], dtype=np.float32)
    key = np.asarray(inputs["key"], dtype=np.float32)
    value = np.asarray(inputs["value"], dtype=np.float32)
    rel_pos_k = np.asarray(inputs["rel_pos_k"], dtype=np.float32)
    rel_pos_v = np.asarray(inputs["rel_pos_v"], dtype=np.float32)

    nc = _get_module(H, S, HD)
    in_maps = _prep_core_inputs(query, key, value, rel_pos_k, rel_pos_v)
    res = run_bass_kernel_spmd(nc, in_maps, core_ids=list(range(NCORES)))

    outv = np.empty((B, S, D), dtype=np.float32)
    for c in range(NCORES):
        outv[c] = _postprocess_core(res.results[c])
    return outv


if __name__ == "__main__":
    rng = np.random.default_rng(0)
    ins = {
        "query": rng.standard_normal((B, S, D)).astype(np.float32),
        "key": rng.standard_normal((B, S, D)).astype(np.float32),
        "value": rng.standard_normal((B, S, D)).astype(np.float32),
        "rel_pos_k": (rng.standard_normal((S, HD)) * 0.02).astype(np.float32),
        "rel_pos_v": (rng.standard_normal((S, HD)) * 0.02).astype(np.float32),
    }
    o = kernel(**ins)
    print("out", o.shape, o.dtype, np.abs(o).max())
